# revision 4
# baseline (speedup 1.0000x reference)
"""Luong dot attention kernel for Trainium2 (Bass/Tile), 8 NeuronCores.

Problem: decoder_output (Q) [8, 2048, 1024] f32, encoder_output (K=V)
[8, 2048, 1024] f32.
  score     = Q @ K^T                  [B, 2048, 2048]
  alignment = softmax(score, axis=-1)
  context   = alignment @ K            [B, 2048, 1024]
Returns (context, alignment) like the reference.

Sharding: data-parallel over batch — core i computes batch i. No
collectives.

Per-core pipeline (all matmuls in fp16 at full PE rate, fp32 accumulate
in PSUM):
  - K, Q cast f32->f16 on load (SWDGE cast-DMA), kept in natural layout.
  - Q^T, K^T built with DMA-transpose (X-bar, 2-byte dtype) so the PE
    only ever runs the two real matmuls.
  - MM1: S[qt] = QT.T @ KT per 512-wide k chunk, accumulated over 8
    d-chunks in PSUM; per-chunk running row-max so softmax starts as
    soon as the last chunk lands.
  - exp on ScalarE with per-row bias=-max, accum_out gives the row sum
    for free; P written as f16.
  - P^T via one DMA-transpose per q-tile; MM2: C = PT.T @ K_nat
    accumulated over 16 k-chunks.
  - alignment = P * (1/sum) (per-partition scalar), context likewise.
"""

import numpy as np

B, TQ, TK, D = 8, 2048, 2048, 1024
P = 128  # partitions
NQT = TQ // P  # 16 q tiles per core
NKT = TK // P  # 16 k tiles
NDC = D // P  # 8 d chunks
KC = 512  # k chunk width for MM1 (one PSUM bank)
NKC = TK // KC  # 4
DC2 = 512  # d chunk width for MM2
NDC2 = D // DC2  # 2

_CACHE = {}


def _patch_tile_drain():
    """Work around a walrus limitation in this container: each instruction
    encoding supports only ONE sync-wait command, but Tile attaches one wait
    per cross-engine dependency. Split extra waits into single-wait NOPs
    emitted just before the instruction on the same engine, and re-emit the
    kernel-tail drain's queue waits as standalone SP wait_ge instructions."""
    import bass_rust
    from concourse.tile import TileContext, ScopedClock

    if getattr(TileContext, "_drain_patched", False):
        return

    _orig_add = TileContext._add_instruction

    def _add_instruction(self, inst):
        si = inst.sync_info
        if si is not None and si.on_wait is not None and len(si.on_wait) > 1:
            waits = list(si.on_wait)
            si.on_wait = waits[-1:]
            for w in waits[:-1]:
                nop = bass_rust.InstNoOp(
                    name=self.nc.get_next_instruction_name(), ins=[], outs=[]
                )
                nop.engine = inst.engine
                nop.sync_info = bass_rust.SyncInfo(on_wait=[w], on_update=[])
                _orig_add(self, nop)
        _orig_add(self, inst)

    TileContext._add_instruction = _add_instruction

    def _drain_and_barrier(self, tick_clock, wait_clock):
        nc = self.nc
        drain_inst = nc.sync.drain()
        wait_clock.add_sem_waits(
            drain_inst.ins, ScopedClock({None: tick_clock.global_clock})
        )
        si = drain_inst.ins.sync_info
        waits = list(si.on_wait or [])
        if len(waits) > 1:
            keep = [w for w in waits if w.wait_mode != "sem-ge-imm"]
            si.on_wait = keep[:1]
            handles = {h.num: h for h in self.sems.allocated().values()}
            for w in waits:
                if w.wait_mode == "sem-ge-imm":
                    nc.sync.wait_ge(handles[w.id], w.wait_value)
        nc.all_engine_barrier()
        popped = nc._tile_sem_poison_stack.pop()
        assert popped is self._sem_poison
        nc.clear_and_free_semaphores(list(self.sems.allocated().values()))
        nc.all_engine_barrier()

    TileContext._drain_and_barrier = _drain_and_barrier
    TileContext._drain_patched = True


def _build():
    import concourse.bass as bass
    import concourse.mybir as mybir
    import concourse.tile as tile

    _patch_tile_drain()

    f32 = mybir.dt.float32
    f16 = mybir.dt.float16

    nc = bass.Bass()
    q_in = nc.dram_tensor("q", [TQ, D], f32, kind="ExternalInput")
    k_in = nc.dram_tensor("k", [TK, D], f32, kind="ExternalInput")
    ctx_out = nc.dram_tensor("ctx", [TQ, D], f32, kind="ExternalOutput")
    align_out = nc.dram_tensor("align", [TQ, TK], f32, kind="ExternalOutput")

    with tile.TileContext(nc) as tc:
        with (
            tc.tile_pool(name="singles", bufs=1) as singles,
            tc.tile_pool(name="p_pool", bufs=2) as p_pool,
            tc.tile_pool(name="pt_pool", bufs=2) as pt_pool,
            tc.tile_pool(name="a_pool", bufs=2) as a_pool,
            tc.tile_pool(name="c_pool", bufs=2) as c_pool,
            tc.tile_pool(name="stat", bufs=4) as stat,
            tc.tile_pool(name="s_ps", bufs=1, space="PSUM") as s_ps_pool,
            tc.tile_pool(name="c_ps", bufs=2, space="PSUM") as c_ps_pool,
        ):
            # ---- phase 0: load + cast + transpose inputs ----
            k16 = singles.tile([P, NKT, D], f16)  # natural K, f16
            q16 = singles.tile([P, NQT, D], f16)  # natural Q, f16
            kT = singles.tile([P, NDC, TK], f16)  # K^T: [dp, dc, k]
            qT = singles.tile([P, NDC, TQ], f16)  # Q^T: [dp, dc, q]

            for t in range(NKT):
                nc.gpsimd.dma_start(
                    out=k16[:, t, :], in_=k_in[t * P : (t + 1) * P, :]
                )
                nc.gpsimd.dma_start(
                    out=q16[:, t, :], in_=q_in[t * P : (t + 1) * P, :]
                )
            for t in range(NKT):
                nc.sync.dma_start_transpose(
                    out=kT[:, :, t * P : (t + 1) * P], in_=k16[:, t, :]
                )
                nc.sync.dma_start_transpose(
                    out=qT[:, :, t * P : (t + 1) * P], in_=q16[:, t, :]
                )

            # ---- main loop over q tiles ----
            prev = None  # deferred MM2 work from previous q tile
            for t in range(NQT):
                s_ps = s_ps_pool.tile([P, NKC, KC], f32)  # 4 banks
                mx4 = stat.tile([P, NKC], f32)
                negm = stat.tile([P, 1], f32)
                ssum = stat.tile([P, 1], f32)
                r = stat.tile([P, 1], f32)

                # MM1: per k-chunk, accumulate 8 d-chunks, then row-max
                for kc in range(NKC):
                    for dc in range(NDC):
                        nc.tensor.matmul(
                            s_ps[:, kc, :],
                            lhsT=qT[:, dc, t * P : (t + 1) * P],
                            rhs=kT[:, dc, kc * KC : (kc + 1) * KC],
                            start=(dc == 0),
                            stop=(dc == NDC - 1),
                        )
                    nc.vector.reduce_max(
                        out=mx4[:, kc : kc + 1],
                        in_=s_ps[:, kc, :],
                        axis=mybir.AxisListType.X,
                    )
                nc.vector.reduce_max(
                    out=negm,
                    in_=mx4,
                    axis=mybir.AxisListType.X,
                    negate=True,
                )

                # softmax numerator (f16) + row sum in one ACT pass
                p16 = p_pool.tile([P, NKC, KC], f16)
                nc.scalar.activation(
                    out=p16,
                    in_=s_ps,
                    func=mybir.ActivationFunctionType.Exp,
                    bias=negm,
                    accum_out=ssum,
                )
                nc.vector.reciprocal(r, ssum)

                # P^T for MM2, one X-bar transpose
                pT = pt_pool.tile([P, NKT, P], f16)
                nc.sync.dma_start_transpose(
                    out=pT, in_=p16.rearrange("p a b -> p (a b)")
                )

                # alignment tile out
                a_t = a_pool.tile([P, TK], f32)
                nc.vector.tensor_scalar_mul(
                    a_t, p16.rearrange("p a b -> p (a b)"), r
                )
                nc.sync.dma_start(
                    out=align_out[t * P : (t + 1) * P, :], in_=a_t
                )

                # defer MM2(t) until after MM1(t+1) so the PE never waits
                # on the P^T transpose DMA
                if prev is not None:
                    _emit_mm2(nc, mybir, c_ps_pool, c_pool, k16, ctx_out, prev)
                prev = (pT, r, t)
            _emit_mm2(nc, mybir, c_ps_pool, c_pool, k16, ctx_out, prev)

    return nc


def _emit_mm2(nc, mybir, c_ps_pool, c_pool, k16, ctx_out, work):
    pT, r, t = work
    f32 = mybir.dt.float32
    c_ps = c_ps_pool.tile([P, NDC2, DC2], f32)  # 2 banks
    for kt in range(NKT):
        for dh in range(NDC2):
            nc.tensor.matmul(
                c_ps[:, dh, :],
                lhsT=pT[:, kt, :],
                rhs=k16[:, kt, dh * DC2 : (dh + 1) * DC2],
                start=(kt == 0),
                stop=(kt == NKT - 1),
            )
    c_t = c_pool.tile([P, D], f32)
    nc.vector.tensor_scalar_mul(
        c_t, c_ps.rearrange("p a b -> p (a b)"), r
    )
    nc.sync.dma_start(out=ctx_out[t * P : (t + 1) * P, :], in_=c_t)


def _get_nc():
    if "nc" not in _CACHE:
        _CACHE["nc"] = _build()
    return _CACHE["nc"]


def kernel(decoder_output, encoder_output):
    from concourse.bass_utils import run_bass_kernel_spmd

    dec = np.ascontiguousarray(np.asarray(decoder_output, dtype=np.float32))
    enc = np.ascontiguousarray(np.asarray(encoder_output, dtype=np.float32))
    nc = _get_nc()
    in_maps = [{"q": dec[i], "k": enc[i]} for i in range(B)]
    res = run_bass_kernel_spmd(nc, in_maps, core_ids=list(range(B)))
    context = np.stack([res.results[i]["ctx"] for i in range(B)])
    alignment = np.stack([res.results[i]["align"] for i in range(B)])
    return context, alignment
